# revision 30
# baseline (speedup 1.0000x reference)
"""AQT int8-quantized matmul (dynamic symmetric quantization) on 8 TRN2 cores.

Full problem: lhs [8192, 4096] f32 @ rhs [4096, 4096] f32 with per-row lhs
scales and per-column rhs scales (abs-max / 127.5), int8 round+clip, int32
matmul, dequantize by the outer product of scales.

Sharding: 2x4 grid over (M, N). Each core gets lhs rows M/2 and rhs cols N/4,
computes its [4096, 1024] output block; host assembles the 8 blocks. Both
quantization axes (lhs rows = per-row over full K, rhs cols = per-column over
full K) keep their full contraction dim on every core, so per-core results
match the unsharded reference exactly. No collectives needed.

Per-core kernel (build_aqt): quantized values are exact integers in
[-127, 127] stored as bf16; TensorE matmul with fp32 PSUM accumulation
reproduces the int32 matmul to ~1e-5. round() is exact round-half-even via
the +1.5*2^23 magic-constant trick (fp32 add/sub). Instead of a post-round
clip, the quant divisor is shrunk by (1-2^-20), which provably keeps rounded
values inside [-127, 127]; dequant uses the same shrunk divisor.

Engine split (mostly as the 694us baseline): DVE = reductions, tensor_tensor,
round tensor_scalar, fused psum-evict scalar_tensor_tensor((psum*s_l)*s_bc);
ScalarE = Abs / scale+round-bias copies; GpSimd = partition_all_reduce and
the output DMAs (software DGE, off the SP queue).

Schedule changes vs the baseline (which lost ~200us to a serial prologue and
to kt-outer/nb-inner matmuls underrunning the streamed rhs quantization):
- lhs m-tiles 0..3 load+quantize early, interleaved with rhs nb0's abs-max
  pass, so their lhsT tiles exist before matmuls begin (~27us).
- matmul ramp: nb0 chains for m-tiles 0..3 start as soon as the first
  quantized nb0 tiles land, joining staggered (a psum chain may visit k in
  any order, so later joiners consume already-resident k-tiles); then the
  same for nb1 while its quantization streams in.
- m-tiles >= 4 run the baseline kt-outer/nb-inner loop with everything
  resident.
"""
import sys

if "/opt/trn_rl_repo" not in sys.path:
    sys.path.insert(0, "/opt/trn_rl_repo")

from contextlib import ExitStack

import numpy as np

from concourse import bacc, bass_isa, mybir, tile
from concourse.bass_utils import run_bass_kernel_spmd

f32 = mybir.dt.float32
bf16 = mybir.dt.bfloat16
Alu = mybir.AluOpType
Act = mybir.ActivationFunctionType

P = 128
C_MAGIC = 1.5 * 2 ** 23
QDIV = 127.5 * (1.0 - 2.0 ** -20)
INV_QDIV = 1.0 / QDIV
TINY = 1e-30

M, K, N = 8192, 4096, 4096
MG, NG = 2, 4                      # shard grid rows (M) x cols (N)
M_loc, N_loc = M // MG, N // NG    # 4096, 1024 per core
N_CORES = MG * NG

NRAMP = 4                          # m-tiles handled by the staggered ramp


def build_aqt(nc, M_loc, K, N_loc, W=512):
    KT, MT, NB = K // P, M_loc // P, N_loc // W

    lhs = nc.declare_dram_parameter("lhs", [M_loc, K], f32, isOutput=False)
    rhs = nc.declare_dram_parameter("rhs", [K, N_loc], f32, isOutput=False)
    out = nc.declare_dram_parameter("out", [M_loc, N_loc], f32, isOutput=True)

    with tile.TileContext(nc) as tc, ExitStack() as ctx:
        pool = lambda name, bufs: ctx.enter_context(tc.tile_pool(name=name, bufs=bufs))
        qr_pool = pool("qr", NB * KT)      # quantized rhs, resident
        sbc_pool = pool("sbc", NB)         # rhs dequant scales, resident
        rstage = pool("rstage", 3)         # rhs raw pass A
        rstage2 = pool("rstage2", 3)       # rhs raw pass B
        rmul = pool("rmul", 3)             # |rhs| / rhs * r_bc
        racc = pool("racc", 2)             # absmax accumulator ping-pong
        rbc = pool("rbc", 2)               # amax_bc / r_bc
        lraw = pool("lraw", 2)             # lhs raw [P, K] f32
        lt1 = pool("lt1", 1)               # lhs scaled+C [P, K] f32
        lqb = pool("lqb", 1)               # lhs quantized [P, K] bf16
        lqt = pool("lqt", NRAMP + 1)       # lhs quantized transposed [P, KT, P]
        lsc = pool("lsc", 1)               # s_l columns, resident
        lam = pool("lam", 4)               # [P, 1] scratch
        opool2 = pool("o2", 3)
        psum = ctx.enter_context(tc.tile_pool(name="psum", bufs=8, space="PSUM"))

        s_l_all = lsc.tile([P, MT], f32)

        qr_tiles = {}
        sbc_tiles = {}
        qt_tiles = {}
        raw_tiles = {}
        racc_state = {}

        # ---------------- rhs helpers (baseline ops) ----------------
        def rhs_A(nb, kt):
            cs = slice(nb * W, (nb + 1) * W)
            t = rstage.tile([P, W], f32, name="rstage")
            nc.sync.dma_start(t[:], rhs[kt * P:(kt + 1) * P, cs])
            ta = rmul.tile([P, W], f32, name="rabs")
            nc.scalar.activation(ta[:], t[:], Act.Abs)
            acc = racc_state.get(nb)
            nacc = racc.tile([P, W], f32, name="racc")
            nc.vector.tensor_tensor(nacc[:], (acc or ta)[:], ta[:], op=Alu.max)
            racc_state[nb] = nacc

        def rhs_scales(nb):
            amax = rbc.tile([P, W], f32, name="amax")
            nc.gpsimd.partition_all_reduce(amax[:], racc_state[nb][:], channels=P,
                                           reduce_op=bass_isa.ReduceOp.absmax)
            s_bc = sbc_pool.tile([P, W], f32, name="sbc")
            nc.vector.tensor_scalar(s_bc[:], amax[:], TINY, INV_QDIV,
                                    op0=Alu.max, op1=Alu.mult)
            sbc_tiles[nb] = s_bc
            r_bc = rbc.tile([P, W], f32, name="rbc")
            nc.vector.reciprocal(r_bc[:], s_bc[:])
            return r_bc

        def rhs_B(nb, kt, r_bc):
            cs = slice(nb * W, (nb + 1) * W)
            t2 = rstage2.tile([P, W], f32, name="rstage2")
            nc.sync.dma_start(t2[:], rhs[kt * P:(kt + 1) * P, cs])
            u = rmul.tile([P, W], f32, name="rmul")
            nc.vector.tensor_tensor(u[:], t2[:], r_bc[:], op=Alu.mult)
            q = qr_pool.tile([P, W], bf16, name="qr")
            nc.vector.tensor_scalar(q[:], u[:], C_MAGIC, C_MAGIC,
                                    op0=Alu.add, op1=Alu.subtract)
            qr_tiles[(nb, kt)] = q

        # ---------------- lhs helpers (baseline ops) ----------------
        def lhs_load(mi):
            raw = lraw.tile([P, K], f32, name="lraw")
            nc.sync.dma_start(raw[:], lhs[mi * P:(mi + 1) * P, :])
            raw_tiles[mi] = raw

        def lhs_quant(mi):
            raw = raw_tiles.pop(mi)
            am = lam.tile([P, 1], f32, name="lam")
            nc.vector.tensor_reduce(am[:], raw[:], axis=mybir.AxisListType.X,
                                    op=Alu.max, apply_absolute_value=True)
            s_col = s_l_all[:, mi:mi + 1]
            nc.vector.tensor_scalar(s_col, am[:], TINY, INV_QDIV,
                                    op0=Alu.max, op1=Alu.mult)
            r_l = lam.tile([P, 1], f32, name="rl")
            nc.vector.reciprocal(r_l[:], s_col)
            t1 = lt1.tile([P, K], f32, name="lt1")
            nc.scalar.activation(t1[:], raw[:], Act.Copy, bias=C_MAGIC, scale=r_l[:])
            qb = lqb.tile([P, K], bf16, name="lqb")
            nc.scalar.activation(qb[:], t1[:], Act.Copy, bias=-C_MAGIC)
            qt = lqt.tile([P, KT, P], bf16, name="lqt")
            # issue from the ACT HWDGE queue: follows the qb ACTIVATE that
            # this engine just produced, so it never parks the SP DMA stream
            nc.scalar.dma_start_transpose(qt[:], qb[:])
            qt_tiles[mi] = qt

        # ---------------- matmul + eviction ----------------
        def evict(mi, nb, ps):
            o2 = opool2.tile([P, W], f32, name="o2")
            nc.vector.scalar_tensor_tensor(
                o2[:], ps[:], s_l_all[:, mi:mi + 1], sbc_tiles[nb][:],
                op0=Alu.mult, op1=Alu.mult)
            nc.gpsimd.dma_start(out[mi * P:(mi + 1) * P, nb * W:(nb + 1) * W],
                                o2[:])

        def ramp(nb, join):
            # staggered-join chains on one nb: chain mi starts at clock
            # join[mi], consuming k-tiles from its join point (a chain may
            # visit k in any order), so TensorE never parks on a tile that
            # the streamed rhs quantization hasn't produced yet
            last = max(join.values()) + KT
            pss = {}
            for c in range(last):
                for mi, j0 in join.items():
                    j = c - j0
                    if not (0 <= j < KT):
                        continue
                    if j == 0:
                        pss[mi] = psum.tile([P, W], f32, name="ps")
                    nc.tensor.matmul(pss[mi][:], qt_tiles[mi][:, j, :],
                                     qr_tiles[(nb, j)][:],
                                     start=(j == 0), stop=(j == KT - 1))
                    if j == KT - 1:
                        evict(mi, nb, pss.pop(mi))

        # ---------------- emission ----------------
        # tile_wait_until floors pin the scheduler's phase order: without
        # them the list scheduler interleaves later-phase DVE work (e.g. nb1
        # abs-max chains) ahead of the qr producers the first matmuls wait on
        with tc.tile_wait_until(0.000):
            lhs_load(0)
            lhs_load(1)
            for kt in range(KT):
                rhs_A(0, kt)
                if kt == 8:
                    lhs_quant(0)
                elif kt == 16:
                    lhs_quant(1)
                elif kt == 20:
                    lhs_load(2)
                elif kt == 28:
                    lhs_quant(2)
        with tc.tile_wait_until(0.040):
            r_bc0 = rhs_scales(0)
        with tc.tile_wait_until(0.042):
            for kt in range(KT):
                rhs_B(0, kt, r_bc0)
                if kt == 8:
                    lhs_load(3)
                elif kt == 16:
                    lhs_quant(3)
        with tc.tile_wait_until(0.044):
            ramp(0, {0: 0, 1: 4, 2: 12, 3: 20})
        with tc.tile_wait_until(0.058):
            for kt in range(KT):
                rhs_A(1, kt)
        with tc.tile_wait_until(0.100):
            r_bc1 = rhs_scales(1)
        with tc.tile_wait_until(0.102):
            for kt in range(KT):
                rhs_B(1, kt, r_bc1)
                if kt == 8:
                    lhs_load(4)
        with tc.tile_wait_until(0.105):
            ramp(1, {0: 0, 1: 4, 2: 10, 3: 16})

        # ---------------- steady state (baseline loop) ----------------
        for mi in range(NRAMP, MT):
            lhs_quant(mi)
            if mi + 1 < MT:
                lhs_load(mi + 1)
            pss = [psum.tile([P, W], f32, name="ps") for _ in range(NB)]
            for kt in range(KT):
                for nb in range(NB):
                    nc.tensor.matmul(pss[nb][:], qt_tiles[mi][:, kt, :],
                                     qr_tiles[(nb, kt)][:],
                                     start=(kt == 0), stop=(kt == KT - 1))
            for nb in range(NB):
                evict(mi, nb, pss[nb])
            del qt_tiles[mi]
    return nc


_COMPILED_NC = None


def _get_compiled():
    global _COMPILED_NC
    if _COMPILED_NC is None:
        nc = bacc.Bacc("TRN2", target_bir_lowering=False, debug=False,
                       num_devices=N_CORES)
        build_aqt(nc, M_loc, K, N_loc)
        nc.compile()
        _COMPILED_NC = nc
    return _COMPILED_NC


def _shard(lhs, rhs):
    in_maps = []
    for i in range(N_CORES):
        mg, ng = divmod(i, NG)
        in_maps.append({
            "lhs": np.ascontiguousarray(lhs[mg * M_loc:(mg + 1) * M_loc, :]),
            "rhs": np.ascontiguousarray(rhs[:, ng * N_loc:(ng + 1) * N_loc]),
        })
    return in_maps


def kernel(lhs, rhs, _trace=False, _trace_kwargs=None):
    lhs = np.asarray(lhs, np.float32)
    rhs = np.asarray(rhs, np.float32)
    nc = _get_compiled()
    res = run_bass_kernel_spmd(nc, _shard(lhs, rhs), core_ids=list(range(N_CORES)),
                               trace=_trace, **(_trace_kwargs or {}))
    out = np.empty((M, N), np.float32)
    for i in range(N_CORES):
        mg, ng = divmod(i, NG)
        out[mg * M_loc:(mg + 1) * M_loc, ng * N_loc:(ng + 1) * N_loc] = \
            res.results[i]["out"]
    kernel.last_result = res
    return out


# revision 31
# speedup vs baseline: 1.0559x; 1.0559x over previous
"""AQT int8-quantized matmul (dynamic symmetric quantization) on 8 TRN2 cores.

Full problem: lhs [8192, 4096] f32 @ rhs [4096, 4096] f32 with per-row lhs
scales and per-column rhs scales (abs-max / 127.5), int8 round+clip, int32
matmul, dequantize by the outer product of scales.

Sharding: 2x4 grid over (M, N). Each core gets lhs rows M/2 and rhs cols N/4,
computes its [4096, 1024] output block; host assembles the 8 blocks. Both
quantization axes (lhs rows = per-row over full K, rhs cols = per-column over
full K) keep their full contraction dim on every core, so per-core results
match the unsharded reference exactly. No collectives needed.

Per-core kernel (build_aqt): quantized values are exact integers in
[-127, 127] stored as bf16; TensorE matmul with fp32 PSUM accumulation
reproduces the int32 matmul to ~1e-5. round() is exact round-half-even via
the +1.5*2^23 magic-constant trick (fp32 add/sub). Instead of a post-round
clip, the quant divisor is shrunk by (1-2^-20), which provably keeps rounded
values inside [-127, 127] and matches the reference's round-then-clip on the
abs-max elements; dequant uses the same shrunk divisor (5e-7 systematic
error). rhs per-column absmax runs as an elementwise max over k-tiles (ACT
Abs + DVE max) followed by one GpSimd partition_all_reduce(absmax). lhs is
quantized in natural [M, K] layout (per-partition row scales on ScalarE),
then moved to [K, M] via DMA-xbar transpose in bf16.

Matmul scheduling: rhs nb=1 tiles stream from the quantizer while the first
matmuls run, so the first m-tiles avoid the baseline's kt-outer/nb-inner
order (which demands both nb blocks per k step and parked TensorE ~60us).
Instead: m-tile 0's nb0 chain runs alone (paced by the nb0 quantizer), then
nb1-paced chains are interleaved k-step by k-step with chains reading the
already-resident nb0 tiles, keeping TensorE fed while nb1 streams in.
m-tiles >= 4 use the baseline kt-outer/nb-inner loop. PSUM eviction is one
fused DVE scalar_tensor_tensor((psum*s_l)*s_bc).
"""
import sys

if "/opt/trn_rl_repo" not in sys.path:
    sys.path.insert(0, "/opt/trn_rl_repo")

from contextlib import ExitStack

import numpy as np

from concourse import bacc, bass_isa, mybir, tile
from concourse.bass_utils import run_bass_kernel_spmd

f32 = mybir.dt.float32
bf16 = mybir.dt.bfloat16
Alu = mybir.AluOpType
Act = mybir.ActivationFunctionType

P = 128
C_MAGIC = 1.5 * 2 ** 23
QDIV = 127.5 * (1.0 - 2.0 ** -20)
INV_QDIV = 1.0 / QDIV
TINY = 1e-30

M, K, N = 8192, 4096, 4096
MG, NG = 2, 4                      # shard grid rows (M) x cols (N)
M_loc, N_loc = M // MG, N // NG    # 4096, 1024 per core
N_CORES = MG * NG


def build_aqt(nc, M_loc, K, N_loc, W=512):
    KT, MT, NB = K // P, M_loc // P, N_loc // W

    lhs = nc.declare_dram_parameter("lhs", [M_loc, K], f32, isOutput=False)
    rhs = nc.declare_dram_parameter("rhs", [K, N_loc], f32, isOutput=False)
    out = nc.declare_dram_parameter("out", [M_loc, N_loc], f32, isOutput=True)

    with tile.TileContext(nc) as tc, ExitStack() as ctx:
        pool = lambda name, bufs: ctx.enter_context(tc.tile_pool(name=name, bufs=bufs))
        qr_pool = pool("qr", NB * KT)      # quantized rhs, resident
        sbc_pool = pool("sbc", NB)         # rhs dequant scales, resident
        rstage = pool("rstage", 4)         # rhs raw pass A
        rstage2 = pool("rstage2", 4)       # rhs raw pass B
        rmul = pool("rmul", 2)             # |rhs| / rhs * r_bc
        racc = pool("racc", 2)             # absmax accumulator ping-pong
        rbc = pool("rbc", 2)               # amax_bc / r_bc
        lraw = pool("lraw", 2)             # lhs raw [P, K] f32
        lt1 = pool("lt1", 1)               # lhs scaled+C [P, K] f32
        lqb = pool("lqb", 1)               # lhs quantized [P, K] bf16
        lqt = pool("lqt", 4)               # lhs quantized transposed [P, KT, P]
        lsc = pool("lsc", 1)               # s_l columns, resident
        lam = pool("lam", 4)               # [P, 1] scratch
        opool2 = pool("o2", 3)
        psum = ctx.enter_context(tc.tile_pool(name="psum", bufs=8, space="PSUM"))

        s_l_all = lsc.tile([P, MT], f32)

        # ---- rhs: absmax -> scales -> quantize (all-resident q_r) ----
        qr_tiles = {}
        sbc_tiles = {}
        for nb in range(NB):
            cs = slice(nb * W, (nb + 1) * W)
            acc = None
            for kt in range(KT):
                t = rstage.tile([P, W], f32, name="rstage")
                nc.sync.dma_start(t[:], rhs[kt * P:(kt + 1) * P, cs])
                ta = rmul.tile([P, W], f32, name="rabs")
                nc.scalar.activation(ta[:], t[:], Act.Abs)
                nacc = racc.tile([P, W], f32, name="racc")
                nc.vector.tensor_tensor(nacc[:], (acc or ta)[:], ta[:], op=Alu.max)
                acc = nacc
            amax = rbc.tile([P, W], f32, name="amax")
            nc.gpsimd.partition_all_reduce(amax[:], acc[:], channels=P,
                                           reduce_op=bass_isa.ReduceOp.absmax)
            s_bc = sbc_pool.tile([P, W], f32, name="sbc")
            nc.vector.tensor_scalar(s_bc[:], amax[:], TINY, INV_QDIV,
                                    op0=Alu.max, op1=Alu.mult)
            sbc_tiles[nb] = s_bc
            r_bc = rbc.tile([P, W], f32, name="rbc")
            nc.vector.reciprocal(r_bc[:], s_bc[:])
            for kt in range(KT):
                t2 = rstage2.tile([P, W], f32, name="rstage2")
                nc.sync.dma_start(t2[:], rhs[kt * P:(kt + 1) * P, cs])
                u = rmul.tile([P, W], f32, name="rmul")
                nc.vector.tensor_tensor(u[:], t2[:], r_bc[:], op=Alu.mult)
                q = qr_pool.tile([P, W], bf16, name="qr")
                nc.vector.tensor_scalar(q[:], u[:], C_MAGIC, C_MAGIC,
                                        op0=Alu.add, op1=Alu.subtract)
                qr_tiles[(nb, kt)] = q

        # ---- lhs quantize + transpose (baseline per-m-tile pipeline) ----
        raw_tiles = {}
        qt_tiles = {}

        def lhs_load(mi):
            raw = lraw.tile([P, K], f32, name="lraw")
            nc.sync.dma_start(raw[:], lhs[mi * P:(mi + 1) * P, :])
            raw_tiles[mi] = raw

        def lhs_quant(mi):
            raw = raw_tiles.pop(mi)
            am = lam.tile([P, 1], f32, name="lam")
            nc.vector.tensor_reduce(am[:], raw[:], axis=mybir.AxisListType.X,
                                    op=Alu.max, apply_absolute_value=True)
            s_col = s_l_all[:, mi:mi + 1]
            nc.vector.tensor_scalar(s_col, am[:], TINY, INV_QDIV,
                                    op0=Alu.max, op1=Alu.mult)
            r_l = lam.tile([P, 1], f32, name="rl")
            nc.vector.reciprocal(r_l[:], s_col)
            t1 = lt1.tile([P, K], f32, name="lt1")
            nc.scalar.activation(t1[:], raw[:], Act.Copy, bias=C_MAGIC, scale=r_l[:])
            qb = lqb.tile([P, K], bf16, name="lqb")
            nc.scalar.activation(qb[:], t1[:], Act.Copy, bias=-C_MAGIC)
            qt = lqt.tile([P, KT, P], bf16, name="lqt")
            nc.sync.dma_start_transpose(qt[:], qb[:])
            qt_tiles[mi] = qt

        def evict(mi, nb, ps):
            o2 = opool2.tile([P, W], f32, name="o2")
            nc.vector.scalar_tensor_tensor(
                o2[:], ps[:], s_l_all[:, mi:mi + 1], sbc_tiles[nb][:],
                op0=Alu.mult, op1=Alu.mult)
            nc.sync.dma_start(out[mi * P:(mi + 1) * P, nb * W:(nb + 1) * W],
                              o2[:])

        def lockstep(pairs):
            # chains advanced one k-tile at a time: pairs reading streamed
            # nb1 tiles interleave with pairs reading resident nb0 tiles, so
            # the in-order TensorE queue always has ready work
            pss = {}
            for pr in pairs:
                pss[pr] = psum.tile([P, W], f32, name="ps")
            for kt in range(KT):
                for mi, nb in pairs:
                    nc.tensor.matmul(pss[(mi, nb)][:], qt_tiles[mi][:, kt, :],
                                     qr_tiles[(nb, kt)][:],
                                     start=(kt == 0), stop=(kt == KT - 1))
            for mi, nb in pairs:
                evict(mi, nb, pss[(mi, nb)])

        # ---- matmuls: ramp on the first 4 m-tiles, then baseline loop ----
        lhs_load(0)
        lhs_load(1)
        lhs_quant(0)
        lhs_load(2)
        lhs_quant(1)
        lhs_load(3)
        lhs_quant(2)

        lockstep([(0, 0)])
        lhs_quant(3)
        lhs_load(4)
        lockstep([(0, 1), (1, 0), (1, 1), (2, 0)])
        lockstep([(2, 1), (3, 0)])
        lockstep([(3, 1)])

        for mi in range(4, MT):
            lhs_quant(mi)
            if mi + 1 < MT:
                lhs_load(mi + 1)
            pss = [psum.tile([P, W], f32, name="ps") for _ in range(NB)]
            for kt in range(KT):
                for nb in range(NB):
                    nc.tensor.matmul(pss[nb][:], qt_tiles[mi][:, kt, :],
                                     qr_tiles[(nb, kt)][:],
                                     start=(kt == 0), stop=(kt == KT - 1))
            for nb in range(NB):
                evict(mi, nb, pss[nb])
            del qt_tiles[mi]
    return nc


_COMPILED_NC = None


def _get_compiled():
    global _COMPILED_NC
    if _COMPILED_NC is None:
        nc = bacc.Bacc("TRN2", target_bir_lowering=False, debug=False,
                       num_devices=N_CORES)
        build_aqt(nc, M_loc, K, N_loc)
        nc.compile()
        _COMPILED_NC = nc
    return _COMPILED_NC


def _shard(lhs, rhs):
    in_maps = []
    for i in range(N_CORES):
        mg, ng = divmod(i, NG)
        in_maps.append({
            "lhs": np.ascontiguousarray(lhs[mg * M_loc:(mg + 1) * M_loc, :]),
            "rhs": np.ascontiguousarray(rhs[:, ng * N_loc:(ng + 1) * N_loc]),
        })
    return in_maps


def kernel(lhs, rhs, _trace=False, _trace_kwargs=None):
    lhs = np.asarray(lhs, np.float32)
    rhs = np.asarray(rhs, np.float32)
    nc = _get_compiled()
    res = run_bass_kernel_spmd(nc, _shard(lhs, rhs), core_ids=list(range(N_CORES)),
                               trace=_trace, **(_trace_kwargs or {}))
    out = np.empty((M, N), np.float32)
    for i in range(N_CORES):
        mg, ng = divmod(i, NG)
        out[mg * M_loc:(mg + 1) * M_loc, ng * N_loc:(ng + 1) * N_loc] = \
            res.results[i]["out"]
    kernel.last_result = res
    return out
